# revision 21
# baseline (speedup 1.0000x reference)
"""Trainium2 Bass kernel for nn_EqStftSnsePBC (STFT -> per-tap nonlinear PBC -> ISTFT).

Strategy (8 NeuronCores, pure data parallelism over STFT frames):
  host:   frame the signal (hop 216, n_fft 256) into [stream, freq_in_frame, frame]
          arrays, shard 4632 (zero-padded) frames as 579 per core; build DFT/IDFT
          lhsT matrices and the 256x256 wrap-around Toeplitz correlation matrix G
          (negated, launch power P and the 1/256 IDFT scale folded in).
  device: computes only the perturbation V = IDFT((j*P*phi) .* X), all in bf16
          (f32 PSUM accumulation):
            X  = DFT(frames)        (matmuls, K=256 contracted in 2x128 chunks)
            I  = sum_modes |X|^2    (DVE)
            na, nb = G @ I          (matmuls; = -P*phi_r, -P*phi_i)
            U  = nb.*X + j(...)     (DVE + GPSIMD elementwise)
            V  = IDFT(U)            (matmuls)
  host:   yf = frames + V (exact passthrough of the linear term), overlap-add,
          divide by coverage count, trim, restack.

Measured on trn2 (8 cores): ~88-91 us HW exec, rel err ~8.8e-3 vs fp32 reference.
"""

import os
import sys

for _p in ("/opt/trn_rl_repo",):
    if os.path.isdir(_p) and _p not in sys.path:
        sys.path.append(_p)

import numpy as np
try:
    import ml_dtypes
    _BF16 = np.dtype(ml_dtypes.bfloat16)
except Exception:
    _BF16 = None

# ---- problem geometry (hardcoded) ----
MTAPS = 41
PAD = MTAPS // 2  # 20
NFFT = 256
HOP = 216
B = 2
NM = 2
L = 999688
STEPS = 4628            # (L - NFFT) // HOP + 1
NCORES = 8
NH = 579                # frames per core
FTOT = NCORES * NH      # 4632 >= STEPS (4 trailing fake frames, ignored on host)
LOUT = L - 2 * PAD * 2  # L - overlaps = 999648
NBLOCKS = ((0, 290), (289, 290))   # (col offset, width); even widths >= 256 (fp32r ISA); 1-col overlap is benign
NSTREAMS = B * NM * 2   # (b, mode, re/im) -> 8

_PROG = None            # (nc, input_names) cache; compile once per process
LAST_EXEC_NS = None
LAST_RESULTS = None


def _build_const_matrices(h_real, h_imag, task_info):
    """DFT/IDFT lhsT matrices and per-batch P-scaled correlation matrices."""
    n = np.arange(NFFT)
    ang = 2.0 * np.pi * np.outer(n, n) / NFFT
    c, s = np.cos(ang), np.sin(ang)
    # forward lhsT[n, f] so that lhsT.T @ frames = W @ frames, W = exp(-2i pi f n / N)
    # stages: 0 fwd_r, 1 fwd_i, 2 fwd_minus_i, 3 inv_r, 4 inv_i, 5 inv_minus_i
    wmat = np.empty((12, 128, NFFT), np.float32)
    stages = [c, -s, s, c / NFFT, s / NFFT, -s / NFFT]
    for st, mat in enumerate(stages):
        wmat[st * 2 + 0] = mat[0:128, :].astype(np.float32)
        wmat[st * 2 + 1] = mat[128:256, :].astype(np.float32)

    # G[p', m] = sum of h[p-m] over p in [0,296) with (p-20) mod 256 == p'
    def toep(h):
        G = np.zeros((NFFT, NFFT), np.float64)
        for p in range(NFFT + 2 * PAD):
            pp = (p - PAD) % NFFT
            lo, hi = max(0, p - (MTAPS - 1)), min(NFFT - 1, p)
            if lo <= hi:
                ms = np.arange(lo, hi + 1)
                G[pp, ms] += h[p - ms]
        return G

    Gr, Gi = toep(np.asarray(h_real, np.float64)), toep(np.asarray(h_imag, np.float64))
    P = (10.0 ** (np.asarray(task_info, np.float64)[:, 0] / 10.0) / NM)
    # negated so the device computes na = -P*phi_r, nb = -P*phi_i directly
    gmat = np.empty((B, 4, 128, NFFT), np.float32)
    for b in range(B):
        for kc in range(2):
            gmat[b, 0 * 2 + kc] = (-P[b] * Gr[kc * 128:(kc + 1) * 128, :]).astype(np.float32)
            gmat[b, 1 * 2 + kc] = (-P[b] * Gi[kc * 128:(kc + 1) * 128, :]).astype(np.float32)
    # packed device layouts: [128, T*256] with T-major columns, all bf16
    wall_packed = np.ascontiguousarray(
        wmat.transpose(1, 0, 2).reshape(128, 12 * NFFT)).astype(_BF16)
    gall_packed = np.ascontiguousarray(
        gmat.reshape(B * 4, 128, NFFT).transpose(1, 0, 2).reshape(128, B * 4 * NFFT)
    ).astype(_BF16)
    return wmat, gmat, wall_packed, gall_packed


def _frame_inputs(x_real, x_imag):
    """-> xf [B, NM, 2, NFFT, FTOT] fp32: xf[b,m,ri,n,j] = x[b, HOP*j + n, m]."""
    need = HOP * (FTOT - 1) + NFFT
    xf = np.empty((B, NM, 2, NFFT, FTOT), np.float32)
    for ri, x in enumerate((x_real, x_imag)):
        xt = np.ascontiguousarray(np.asarray(x).transpose(0, 2, 1))  # [B, NM, L]
        xp = np.zeros((B, NM, need), np.float32)
        xp[:, :, :L] = xt
        sw = np.lib.stride_tricks.as_strided(
            xp,
            shape=(B, NM, FTOT, NFFT),
            strides=(xp.strides[0], xp.strides[1], HOP * 4, 4),
        )
        xf[:, :, ri] = sw.transpose(0, 1, 3, 2)
    return xf


def _overlap_add(yf):
    """yf [B, NM, 2, NFFT, FTOT] -> y [B, NM, 2, L] (OLA / coverage)."""
    y = np.zeros((B, NM, 2, STEPS, HOP), np.float32)
    body = yf[:, :, :, :HOP, :STEPS].transpose(0, 1, 2, 4, 3)  # [.., j, 216]
    y[:] = body
    tail = yf[:, :, :, HOP:, :STEPS - 1].transpose(0, 1, 2, 4, 3)  # [.., j, 40]
    y[:, :, :, 1:, :NFFT - HOP] += tail
    y = y.reshape(B, NM, 2, STEPS * HOP)
    yfull = np.empty((B, NM, 2, L), np.float32)
    yfull[:, :, :, :STEPS * HOP] = y
    yfull[:, :, :, STEPS * HOP:] = yf[:, :, :, HOP:, STEPS - 1]  # final tail, coverage 1
    # coverage: 2 on [216(j+1), 216(j+1)+40) for j=0..steps-2, else 1
    t = np.arange(L)
    wsum = np.ones(L, np.float32)
    wsum[(t >= HOP) & (t < STEPS * HOP) & (t % HOP < NFFT - HOP)] = 2.0
    yfull /= wsum
    return yfull


def _build_program():
    import concourse.bass as bass
    import concourse.tile as tile
    from concourse import bacc, mybir
    from contextlib import ExitStack

    f32 = mybir.dt.float32
    f32r = mybir.dt.float32r
    bf16 = mybir.dt.bfloat16
    MULT = mybir.AluOpType.mult
    ADD = mybir.AluOpType.add
    SUB = mybir.AluOpType.subtract

    nc = bacc.Bacc(None, target_bir_lowering=False, debug=False)
    xf_d = nc.dram_tensor("xf", [NSTREAMS, NFFT, NH], bf16, kind="ExternalInput").ap()
    wall_d = nc.dram_tensor("wall", [128, 12 * NFFT], bf16, kind="ExternalInput").ap()
    gmb_d = nc.dram_tensor("gmatb", [128, B * 4 * NFFT], bf16, kind="ExternalInput").ap()
    vf_d = nc.dram_tensor("vf", [NSTREAMS, NFFT, NH], bf16, kind="ExternalOutput").ap()

    FWD_R, FWD_I, FWD_MI, INV_R, INV_I, INV_MI = range(6)
    FFT_TERMS = {0: ((FWD_R, 0), (FWD_MI, 1)),   # Xr = Wr xr - Wi xi
                 1: ((FWD_R, 1), (FWD_I, 0))}    # Xi = Wr xi + Wi xr
    IFFT_TERMS = {0: ((INV_R, 0), (INV_MI, 1)),  # Vr = iWr Ur - iWi Ui
                  1: ((INV_R, 1), (INV_I, 0))}   # Vi = iWr Ui + iWi Ur

    BLOCKS = [(b, j0, NB) for b in range(B) for (j0, NB) in NBLOCKS]

    with tile.TileContext(nc) as tc:
        with ExitStack() as ctx:
            consts = ctx.enter_context(tc.tile_pool(name="consts", bufs=1))
            NBUFS = int(os.environ.get("K_BUFS", "3"))
            xin = ctx.enter_context(tc.tile_pool(name="xin", bufs=NBUFS))
            xcp = ctx.enter_context(tc.tile_pool(name="xcp", bufs=NBUFS))
            work = ctx.enter_context(tc.tile_pool(name="work", bufs=NBUFS))
            usb_p = ctx.enter_context(tc.tile_pool(name="usb", bufs=NBUFS))
            osb_p = ctx.enter_context(tc.tile_pool(name="osb", bufs=NBUFS))
            ps_x = ctx.enter_context(tc.tile_pool(name="psx", bufs=4, space="PSUM"))
            ps_phi = ctx.enter_context(tc.tile_pool(name="psphi", bufs=2, space="PSUM"))
            ps_v = ctx.enter_context(tc.tile_pool(name="psv", bufs=2, space="PSUM"))

            # constants, consolidated into 3 DMAs (startup latency)
            wall = consts.tile([128, 12 * NFFT], bf16, tag="wall")
            nc.sync.dma_start(wall[:], wall_d[:])
            gall = consts.tile([128, B * 4 * NFFT], bf16, tag="gall")
            nc.scalar.dma_start(gall[:], gmb_d[:])
            wsb = {}
            for st in range(6):
                for kc in range(2):
                    for mh in range(2):
                        off = (st * 2 + kc) * NFFT + mh * 128
                        wsb[(st, kc, mh)] = wall[:, off:off + 128]
            gsb = {}
            for b in range(B):
                for t in range(4):
                    for mh in range(2):
                        off = (b * 4 + t) * NFFT + mh * 128
                        gsb[(b, t, mh)] = gall[:, off:off + 128]

            def sidx(b, m, ri):
                return b * 4 + m * 2 + ri

            state = {}

            def emit_load_fft(t):
                """DMA frames in, FFT (f32r, m-paired weights), cast X to bf16,
                and intensity — fills state[t]."""
                b, j0, NB = BLOCKS[t]
                xsb = {}
                for m in range(NM):
                    for ri in range(2):
                        for kc in range(2):
                            tl = xin.tile([128, NB], bf16, tag=f"x{m}{ri}{kc}",
                                          name=f"x{t}_{m}{ri}{kc}")
                            nc.sync.dma_start(
                                tl[:],
                                xf_d[sidx(b, m, ri), kc * 128:(kc + 1) * 128,
                                     j0:j0 + NB],
                            )
                            xsb[(m, ri, kc)] = tl
                Xsb = {}
                for ri_o in range(2):
                    for mh in range(2):
                        xps = [ps_x.tile([128, NB], f32, tag="xps",
                                         name=f"xps{t}_{ri_o}{mh}{_m}") for _m in range(NM)]
                        seq = [(st, src, kc)
                               for (st, src) in FFT_TERMS[ri_o] for kc in range(2)]
                        if PAIRED:
                            for i, (st, src, kc) in enumerate(seq):
                                for m in range(NM):
                                    nc.tensor.matmul(
                                        xps[m][:], wsb[(st, kc, mh)],
                                        xsb[(m, src, kc)][:],
                                        start=(i == 0), stop=(i == len(seq) - 1),
                                    )
                        else:
                            for m in range(NM):
                                for i, (st, src, kc) in enumerate(seq):
                                    nc.tensor.matmul(
                                        xps[m][:], wsb[(st, kc, mh)],
                                        xsb[(m, src, kc)][:],
                                        start=(i == 0), stop=(i == len(seq) - 1),
                                    )
                        xs = xcp.tile([128, 2 * NB], bf16, tag=f"X{ri_o}{mh}",
                                      name=f"X{t}_{ri_o}{mh}")
                        for m in range(NM):
                            dst = xs[:, m * NB:(m + 1) * NB]
                            if (m + ri_o) % 2 == 0:
                                nc.vector.tensor_copy(dst, xps[m][:])
                            else:
                                nc.scalar.copy(dst, xps[m][:])
                        Xsb[(ri_o, mh)] = xs
                isb = {}
                for mh in range(2):
                    it = work.tile([128, NB], bf16, tag=f"i{mh}", name=f"i{t}_{mh}")
                    t0 = work.tile([128, 2 * NB], bf16, tag=f"sqa{mh}", name=f"sqa{t}_{mh}")
                    t1 = work.tile([128, 2 * NB], bf16, tag=f"sqb{mh}", name=f"sqb{t}_{mh}")
                    nc.vector.tensor_tensor(t0[:], Xsb[(0, mh)][:], Xsb[(0, mh)][:], MULT)
                    nc.vector.tensor_tensor(t1[:], Xsb[(1, mh)][:], Xsb[(1, mh)][:], MULT)
                    nc.vector.tensor_tensor(t0[:], t0[:], t1[:], ADD)
                    nc.vector.tensor_tensor(it[:], t0[:, 0:NB], t0[:, NB:2 * NB], ADD)
                    isb[mh] = it
                state[t] = {"xsb": xsb, "Xsb": Xsb, "isb": isb}

            def emit_corr_u(t):
                """corr matmuls, phi copies, U elementwise for block t."""
                b, j0, NB = BLOCKS[t]
                st_ = state[t]
                nab = {}
                for mh in range(2):
                    for ri in range(2):
                        pp = ps_phi.tile([128, NB], f32, tag="phps",
                                         name=f"ph{t}_{ri}{mh}")
                        for kc in range(2):
                            nc.tensor.matmul(
                                pp[:], gsb[(b, ri * 2 + kc, mh)], st_["isb"][kc][:],
                                start=(kc == 0), stop=(kc == 1),
                            )
                        ab = work.tile([128, 2 * NB], bf16, tag=f"ab{ri}{mh}",
                                       name=f"ab{t}_{ri}{mh}")
                        nc.scalar.copy(ab[:, 0:NB], pp[:])
                        nc.scalar.copy(ab[:, NB:2 * NB], pp[:])
                        nab[(ri, mh)] = ab
                usb = {}
                for mh in range(2):
                    na, nb_ = nab[(0, mh)], nab[(1, mh)]
                    Xr, Xi = st_["Xsb"][(0, mh)], st_["Xsb"][(1, mh)]
                    t0 = work.tile([128, 2 * NB], bf16, tag=f"ut0{mh}", name=f"ut0{t}_{mh}")
                    t1 = work.tile([128, 2 * NB], bf16, tag=f"ut1{mh}", name=f"ut1{t}_{mh}")
                    t2 = work.tile([128, 2 * NB], bf16, tag=f"ut2{mh}", name=f"ut2{t}_{mh}")
                    t3 = work.tile([128, 2 * NB], bf16, tag=f"ut3{mh}", name=f"ut3{t}_{mh}")
                    ur = usb_p.tile([128, 2 * NB], bf16, tag=f"ur{mh}", name=f"ur{t}_{mh}")
                    ui = usb_p.tile([128, 2 * NB], bf16, tag=f"ui{mh}", name=f"ui{t}_{mh}")
                    # muls split GPS/DVE so the two operands of each add run in parallel
                    nc.gpsimd.tensor_tensor(t0[:], nb_[:], Xr[:], MULT)
                    nc.vector.tensor_tensor(t1[:], na[:], Xi[:], MULT)
                    nc.gpsimd.tensor_tensor(t2[:], nb_[:], Xi[:], MULT)
                    nc.vector.tensor_tensor(t3[:], na[:], Xr[:], MULT)
                    nc.vector.tensor_tensor(ur[:], t0[:], t1[:], ADD)
                    nc.vector.tensor_tensor(ui[:], t2[:], t3[:], SUB)
                    usb[(0, mh)] = ur
                    usb[(1, mh)] = ui
                st_["usb"] = usb

            def emit_ifft(t):
                b, j0, NB = BLOCKS[t]
                usb = state[t]["usb"]
                for ri_o in range(2):
                    for nh in range(2):
                        vps = [ps_v.tile([128, NB], f32, tag="vps",
                                         name=f"vps{t}_{ri_o}{nh}{_m}") for _m in range(NM)]
                        seq = [(st, src, kc)
                               for kc in (1, 0) for (st, src) in IFFT_TERMS[ri_o]]
                        for i, (st, src, kc) in enumerate(seq):
                            for m in range(NM):
                                nc.tensor.matmul(
                                    vps[m][:], wsb[(st, kc, nh)],
                                    usb[(src, kc)][:, m * NB:(m + 1) * NB],
                                    start=(i == 0), stop=(i == len(seq) - 1),
                                )
                        for m in range(NM):
                            ob = osb_p.tile([128, NB], bf16, tag=f"o{m}{ri_o}{nh}",
                                            name=f"o{t}_{m}{ri_o}{nh}")
                            nc.scalar.copy(ob[:], vps[m][:])
                            nc.sync.dma_start(
                                vf_d[sidx(b, m, ri_o), nh * 128:(nh + 1) * 128,
                                     j0:j0 + NB],
                                ob[:],
                            )
                del state[t]

            # software pipeline: corr/U of block t overlaps FFT of block t+1
            PIPELINE = os.environ.get("K_PIPELINE", "1") == "1"
            PAIRED = os.environ.get("K_PAIR", "1") == "1"
            if PIPELINE:
                emit_load_fft(0)
                for t in range(len(BLOCKS)):
                    emit_corr_u(t)
                    if t + 1 < len(BLOCKS):
                        emit_load_fft(t + 1)
                    emit_ifft(t)
            else:
                for t in range(len(BLOCKS)):
                    emit_load_fft(t)
                    emit_corr_u(t)
                    emit_ifft(t)

    nc.compile()
    return nc


def _run_device(xf, wall_packed, gall_packed, trace=False):
    """xf [B,NM,2,NFFT,FTOT] -> vf same shape, via 8-core SPMD bass kernel."""
    global _PROG, LAST_EXEC_NS, LAST_RESULTS
    from concourse.bass_utils import run_bass_kernel_spmd

    if _PROG is None:
        _PROG = _build_program()
    nc = _PROG

    xfs = xf.reshape(NSTREAMS, NFFT, FTOT).astype(_BF16)
    in_maps = []
    for k in range(NCORES):
        in_maps.append({
            "xf": np.ascontiguousarray(xfs[:, :, k * NH:(k + 1) * NH]),
            "wall": wall_packed,
            "gmatb": gall_packed,
        })
    kwargs = {}
    if trace:
        kwargs["trace"] = True
    res = run_bass_kernel_spmd(nc, in_maps, list(range(NCORES)), **kwargs)
    LAST_EXEC_NS = res.exec_time_ns
    LAST_RESULTS = res
    vf = np.empty((NSTREAMS, NFFT, FTOT), np.float32)
    for k in range(NCORES):
        vf[:, :, k * NH:(k + 1) * NH] = res.results[k]["vf"].astype(np.float32)
    return vf.reshape(B, NM, 2, NFFT, FTOT)


def _emulate_device(xf, wmat, gmat):
    """Numpy mirror of the device program: returns V = IFFT(j P phi * X)."""
    W = {st: np.concatenate([wmat[st * 2], wmat[st * 2 + 1]], 0) for st in range(6)}
    vf = np.empty_like(xf)
    for b in range(B):
        G = {ri: np.concatenate([gmat[b, ri * 2], gmat[b, ri * 2 + 1]], 0) for ri in range(2)}
        Xr = np.einsum('nf,mnj->mfj', W[0], xf[b, :, 0]) + np.einsum('nf,mnj->mfj', W[2], xf[b, :, 1])
        Xi = np.einsum('nf,mnj->mfj', W[0], xf[b, :, 1]) + np.einsum('nf,mnj->mfj', W[1], xf[b, :, 0])
        I = (Xr * Xr + Xi * Xi).sum(axis=0)
        na = G[0].T @ I    # = -P*phi_r
        nb = G[1].T @ I    # = -P*phi_i
        Ur, Ui = nb * Xr + na * Xi, nb * Xi - na * Xr
        vf[b, :, 0] = np.einsum('fn,mfj->mnj', W[3], Ur) + np.einsum('fn,mfj->mnj', W[5], Ui)
        vf[b, :, 1] = np.einsum('fn,mfj->mnj', W[3], Ui) + np.einsum('fn,mfj->mnj', W[4], Ur)
    return vf


def kernel(x_real, x_imag, task_info, h_real, h_imag, _emulate=False, _trace=False):
    x_real = np.asarray(x_real, np.float32)
    x_imag = np.asarray(x_imag, np.float32)
    wmat, gmat, wall_packed, gall_packed = _build_const_matrices(h_real, h_imag, task_info)
    xf = _frame_inputs(x_real, x_imag)
    if _emulate:
        vf = _emulate_device(xf, wmat, gmat)
    else:
        vf = _run_device(xf, wall_packed, gall_packed, trace=_trace)
    yf = xf + vf                              # exact passthrough + device correction
    y = _overlap_add(yf)                      # [B, NM, 2, L]
    y = y[:, :, :, PAD:L - PAD]               # trim overlaps//2 each side
    return np.ascontiguousarray(y.transpose(0, 3, 1, 2))  # [B, LOUT, NM, 2]
